# revision 1
# baseline (speedup 1.0000x reference)
"""EnsRec loss kernel for 8 Trainium2 NeuronCores.

Data-parallel over batch (64 rows per core); item/user tables and W_proj
replicated. Algebraic restructuring vs the reference:
  basemodel_emb = (sum_l tw[l]*mask*E[ids]) @ W_proj + b_proj*sum(tw)
(projection commutes with the time-decay sum, and the validity mask is
folded into the per-(bk,l) weight so id==0 rows need no table edit).
Each core emits per-row partial losses; the host does the final 8-way sum.
"""

import sys

import numpy as np

_TRN_REPO = "/opt/trn_rl_repo"
if _TRN_REPO not in sys.path:
    sys.path.insert(0, _TRN_REPO)

import concourse.bacc as bacc
import concourse.mybir as mybir
import concourse.tile as tile
from concourse.bass import IndirectOffsetOnAxis
from concourse.bass_utils import run_bass_kernel_spmd

B, K, L, D, H = 512, 8, 50, 768, 128
N_ITEM = 100000
N_USER = 50000
DIV_TRADEOFF = 0.1
NCORES = 8
BLOC = B // NCORES          # 64 batch rows per core
BK = BLOC * K               # 512 (b,k) rows per core
NCHUNK = BK // 128          # 4 partition-tiles of (b,k) rows
LG = 10                     # seq positions gathered per indirect DMA
NG = L // LG                # 5 gather groups per chunk
OUT_LEN = 2 * BLOC

_f32 = mybir.dt.float32
_bf16 = mybir.dt.bfloat16
_i32 = mybir.dt.int32
ALU = mybir.AluOpType
AFT = mybir.ActivationFunctionType
AXL = mybir.AxisListType

_CACHED_NC = None


def _build_module():
    nc = bacc.Bacc("TRN2", target_bir_lowering=False, debug=False,
                   num_devices=NCORES)

    table = nc.dram_tensor("table", [N_ITEM + 1, D], _bf16, kind="ExternalInput")
    utable = nc.dram_tensor("utable", [N_USER, H], _f32, kind="ExternalInput")
    wproj = nc.dram_tensor("wproj", [D, H], _f32, kind="ExternalInput")
    beff = nc.dram_tensor("beff", [H, 1], _f32, kind="ExternalInput")
    bmask = nc.dram_tensor("bmask", [128, 128], _f32, kind="ExternalInput")
    identin = nc.dram_tensor("identin", [128, 128], _f32, kind="ExternalInput")
    idx = nc.dram_tensor("idx", [128, NCHUNK * L], _i32, kind="ExternalInput")
    wm = nc.dram_tensor("wm", [128, NCHUNK * L], _f32, kind="ExternalInput")
    uid = nc.dram_tensor("uid", [BLOC, 1], _i32, kind="ExternalInput")
    prefin = nc.dram_tensor("prefin", [BLOC, H], _f32, kind="ExternalInput")
    posT = nc.dram_tensor("posT", [1, BK], _f32, kind="ExternalInput")
    negT = nc.dram_tensor("negT", [1, BK], _f32, kind="ExternalInput")
    out = nc.dram_tensor("out", [OUT_LEN], _f32, kind="ExternalOutput")
    wscr1 = nc.dram_tensor("wscr1", [BK], _f32)

    with tile.TileContext(nc) as tc:
        with (
            tc.tile_pool(name="big", bufs=44) as bigp,
            tc.tile_pool(name="sb", bufs=1) as sbp,
            tc.tile_pool(name="work", bufs=2) as workp,
            tc.tile_pool(name="ps2", bufs=2, space="PSUM") as ps2,
            tc.tile_pool(name="ps1", bufs=1, space="PSUM") as ps1,
        ):
            ident = sbp.tile([128, 128], _f32, tag="ident")
            nc.sync.dma_start(out=ident[:], in_=identin[:])
            idx_sb = sbp.tile([128, NCHUNK * L], _i32, tag="idx")
            nc.sync.dma_start(out=idx_sb[:], in_=idx[:])
            wm_sb = sbp.tile([128, NCHUNK * L], _f32, tag="wm")
            nc.sync.dma_start(out=wm_sb[:], in_=wm[:])
            wall = sbp.tile([128, 6 * 128], _f32, tag="wall")
            for c in range(6):
                nc.sync.dma_start(out=wall[:, c * 128:(c + 1) * 128],
                                  in_=wproj[c * 128:(c + 1) * 128, :])
            beff_sb = sbp.tile([H, 1], _f32, tag="beff")
            nc.sync.dma_start(out=beff_sb[:], in_=beff[:])
            bmask_sb = sbp.tile([128, 128], _f32, tag="bmask")
            nc.sync.dma_start(out=bmask_sb[:], in_=bmask[:])
            warm = sbp.tile([1, 1], _f32, tag="warm")
            nc.vector.memset(warm[:], 1.0)
            nc.scalar.activation(out=warm[:], in_=warm[:], func=AFT.Exp)
            nc.scalar.activation(out=warm[:], in_=warm[:], func=AFT.Ln,
                                 bias=1.0)

            # ---- preference = prefin + utable[uid], transposed+replicated ----
            uid_sb = sbp.tile([BLOC, 1], _i32, tag="uid")
            nc.sync.dma_start(out=uid_sb[:], in_=uid[:])
            pref = sbp.tile([BLOC, H], _f32, tag="pref")
            nc.gpsimd.indirect_dma_start(
                out=pref[:], out_offset=None, in_=utable[:],
                in_offset=IndirectOffsetOnAxis(ap=uid_sb[:, :1], axis=0))
            prefin_sb = sbp.tile([BLOC, H], _f32, tag="prefin")
            nc.sync.dma_start(out=prefin_sb[:], in_=prefin[:])
            nc.vector.tensor_tensor(out=pref[:], in0=pref[:], in1=prefin_sb[:],
                                    op=ALU.add)
            ptp = ps1.tile([128, BLOC], _f32, tag="ptp")
            nc.tensor.transpose(out=ptp[:], in_=pref[:],
                                identity=ident[:BLOC, :BLOC])
            prep = sbp.tile([128, 512], _f32, tag="prep")
            prep3 = prep[:].rearrange("p (b k) -> p b k", k=K)
            for k in range(K):
                nc.vector.tensor_copy(out=prep3[:, :, k], in_=ptp[:])

            # ---- main gather + weighted accumulate + transpose ----
            # HW indirect DMA consumes exactly one index per partition, so
            # each gather moves 128 table rows (one per (b,k) row).
            wsumT = sbp.tile([128, 6 * 512], _f32, tag="wsumT")
            eT = sbp.tile([128, 512], _f32, tag="eT")
            r_all = sbp.tile([128, NCHUNK], _f32, tag="rall")
            ones = sbp.tile([128, 1], _f32, tag="ones")
            nc.vector.memset(ones[:], 1.0)
            wop = ps1.tile([1, 512], _f32, tag="wop")
            for t in range(NCHUNK):
                acc = workp.tile([128, D], _f32, tag="acc")
                for l in range(L):
                    col = t * L + l
                    gt = bigp.tile([128, D], _bf16, tag="gath")
                    nc.gpsimd.indirect_dma_start(
                        out=gt[:],
                        out_offset=None,
                        in_=table[:],
                        in_offset=IndirectOffsetOnAxis(
                            ap=idx_sb[:, col:col + 1], axis=0),
                    )
                    if l == 0:
                        nc.vector.tensor_scalar(
                            out=acc[:], in0=gt[:],
                            scalar1=wm_sb[:, col:col + 1], scalar2=None,
                            op0=ALU.mult)
                    else:
                        nc.vector.scalar_tensor_tensor(
                            out=acc[:], in0=gt[:],
                            scalar=wm_sb[:, col:col + 1], in1=acc[:],
                            op0=ALU.mult, op1=ALU.add)
                # per-chunk tail: transpose, project, gram, score — so only
                # the final chunk's ops sit on the post-gather critical path
                for c in range(6):
                    tp = ps2.tile([128, 128], _f32, tag="tp")
                    nc.tensor.transpose(out=tp[:],
                                        in_=acc[:, c * 128:(c + 1) * 128],
                                        identity=ident[:])
                    nc.vector.tensor_copy(
                        out=wsumT[:, c * 512 + t * 128: c * 512 + (t + 1) * 128],
                        in_=tp[:])
                eTp = ps2.tile([128, 128], _f32, tag="eTp")
                for c in range(6):
                    nc.tensor.matmul(
                        out=eTp[:],
                        lhsT=wall[:, c * 128:(c + 1) * 128],
                        rhs=wsumT[:, c * 512 + t * 128: c * 512 + (t + 1) * 128],
                        start=(c == 0), stop=(c == 5))
                nc.vector.tensor_scalar(out=eT[:, t * 128:(t + 1) * 128],
                                        in0=eTp[:], scalar1=beff_sb[:],
                                        scalar2=None, op0=ALU.add)
                sp = ps2.tile([128, 128], _f32, tag="sp")
                nc.tensor.matmul(out=sp[:], lhsT=eT[:, t * 128:(t + 1) * 128],
                                 rhs=eT[:, t * 128:(t + 1) * 128],
                                 start=True, stop=True)
                spc = workp.tile([128, 128], _f32, tag="spc")
                nc.vector.tensor_copy(out=spc[:], in_=sp[:])
                s2 = workp.tile([128, 128], _f32, tag="s2")
                nc.vector.tensor_tensor(out=s2[:], in0=sp[:], in1=spc[:],
                                        op=ALU.mult)
                dummy = workp.tile([128, 128], _f32, tag="dummy")
                nc.vector.scalar_tensor_tensor(
                    out=dummy[:], in0=s2[:], scalar=1.0, in1=bmask_sb[:],
                    op0=ALU.mult, op1=ALU.mult, accum_out=r_all[:, t:t + 1])
                prod = workp.tile([128, 128], _f32, tag="prod")
                nc.vector.tensor_tensor(out=prod[:],
                                        in0=eT[:, t * 128:(t + 1) * 128],
                                        in1=prep[:, t * 128:(t + 1) * 128],
                                        op=ALU.mult)
                nc.tensor.matmul(out=wop[:, t * 128:(t + 1) * 128],
                                 lhsT=ones[:], rhs=prod[:],
                                 start=True, stop=True)

            # ---- tail entirely in the [1, BK] row layout (col = b*K + k) ----
            # u = exp(worg) (no max-sub: |worg| <~ 6); softmax normalization is
            # folded into the pu/nu and div ratios, so no k-broadcast needed.
            u = sbp.tile([1, BK], _f32, tag="u")
            nc.scalar.activation(out=u[:], in_=wop[:], func=AFT.Exp)
            u3 = u[:].rearrange("o (b k) -> o b k", k=K)
            s = sbp.tile([1, BLOC], _f32, tag="s")
            nc.vector.tensor_reduce(out=s[:], in_=u3, axis=AXL.X, op=ALU.add)
            rs = sbp.tile([1, BLOC], _f32, tag="rs")
            nc.vector.reciprocal(out=rs[:], in_=s[:])

            pos_sb = sbp.tile([1, BK], _f32, tag="pos")
            nc.sync.dma_start(out=pos_sb[:], in_=posT[:])
            neg_sb = sbp.tile([1, BK], _f32, tag="neg")
            nc.sync.dma_start(out=neg_sb[:], in_=negT[:])
            pu = sbp.tile([1, BK], _f32, tag="pu")
            nc.vector.tensor_tensor(out=pu[:], in0=pos_sb[:], in1=u[:],
                                    op=ALU.mult)
            pug = sbp.tile([1, BLOC], _f32, tag="pug")
            nc.vector.tensor_reduce(out=pug[:],
                                    in_=pu[:].rearrange("o (b k) -> o b k", k=K),
                                    axis=AXL.X, op=ALU.add)
            nu = sbp.tile([1, BK], _f32, tag="nu")
            nc.vector.tensor_tensor(out=nu[:], in0=neg_sb[:], in1=u[:],
                                    op=ALU.mult)
            nug = sbp.tile([1, BLOC], _f32, tag="nug")
            nc.vector.tensor_reduce(out=nug[:],
                                    in_=nu[:].rearrange("o (b k) -> o b k", k=K),
                                    axis=AXL.X, op=ALU.add)
            dnum = sbp.tile([1, BLOC], _f32, tag="dnum")
            nc.vector.tensor_tensor(out=dnum[:], in0=pug[:], in1=nug[:],
                                    op=ALU.subtract)
            dlt = sbp.tile([1, BLOC], _f32, tag="dlt")
            nc.vector.tensor_tensor(out=dlt[:], in0=dnum[:], in1=rs[:],
                                    op=ALU.mult)
            expt = sbp.tile([1, BLOC], _f32, tag="expt")
            nc.scalar.activation(out=expt[:], in_=dlt[:], func=AFT.Exp,
                                 scale=-1.0)
            bce = sbp.tile([1, BLOC], _f32, tag="bce")
            nc.scalar.activation(out=bce[:], in_=expt[:], func=AFT.Ln,
                                 bias=1.0)
            nc.sync.dma_start(out=out[None, 0:BLOC], in_=bce[:])

            # ---- div part: bounce r_all to the row layout, then
            # out[64:128] = per-b sum_k u*r / s ----
            rtp = ps1.tile([NCHUNK, 128], _f32, tag="ptp")
            nc.tensor.transpose(out=rtp[:], in_=r_all[:], identity=ident[:])
            rts = sbp.tile([NCHUNK, 128], _f32, tag="rts")
            nc.vector.tensor_copy(out=rts[:], in_=rtp[:])
            nc.scalar.dma_start(out=wscr1[:].rearrange("(t p) -> t p", p=128),
                                in_=rts[:])
            rrow = sbp.tile([1, BK], _f32, tag="rrow")
            nc.scalar.dma_start(out=rrow[:], in_=wscr1[None, :])
            ur = sbp.tile([1, BK], _f32, tag="ur")
            nc.vector.tensor_tensor(out=ur[:], in0=u[:], in1=rrow[:],
                                    op=ALU.mult)
            urg = sbp.tile([1, BLOC], _f32, tag="urg")
            nc.vector.tensor_reduce(out=urg[:],
                                    in_=ur[:].rearrange("o (b k) -> o b k", k=K),
                                    axis=AXL.X, op=ALU.add)
            dvb = sbp.tile([1, BLOC], _f32, tag="dvb")
            nc.vector.tensor_tensor(out=dvb[:], in0=urg[:], in1=rs[:],
                                    op=ALU.mult)
            nc.sync.dma_start(out=out[None, BLOC:], in_=dvb[:])

    nc.compile()
    return nc


def _get_nc():
    global _CACHED_NC
    if _CACHED_NC is None:
        _CACHED_NC = _build_module()
    return _CACHED_NC


def _prep_in_maps(user_id, base_model_preds, preference_in, pos_label,
                  neg_label, user_embeddings, item_embeddings, W_proj, b_proj):
    tw = (1.0 / np.log2(np.arange(L, dtype=np.float32) + 2.0)).astype(np.float32)
    import ml_dtypes
    table = np.ascontiguousarray(
        np.asarray(item_embeddings, dtype=np.float32).astype(ml_dtypes.bfloat16))
    utable = np.ascontiguousarray(np.asarray(user_embeddings, dtype=np.float32))
    wproj = np.ascontiguousarray(np.asarray(W_proj, dtype=np.float32))
    beff = (np.asarray(b_proj, dtype=np.float32) * np.float32(tw.sum())
            ).reshape(H, 1)
    ident_np = np.eye(128, dtype=np.float32)
    bmask = (np.kron(np.eye(16, dtype=np.float32),
                     np.ones((8, 8), dtype=np.float32))
             - np.eye(128, dtype=np.float32)).astype(np.float32)

    preds = np.asarray(base_model_preds).astype(np.int64)
    uid_all = np.asarray(user_id).astype(np.int32).reshape(B, 1)
    pref_all = np.asarray(preference_in, dtype=np.float32)
    pos_all = np.asarray(pos_label, dtype=np.float32)
    neg_all = np.asarray(neg_label, dtype=np.float32)

    in_maps = []
    for c in range(NCORES):
        s = slice(c * BLOC, (c + 1) * BLOC)
        pf = preds[s].reshape(BK, L)                       # [512, 50]
        valid = (pf > 0) & (pf <= N_ITEM)
        safe = np.where(valid, pf, 0).astype(np.int32)
        idx = np.ascontiguousarray(
            safe.reshape(NCHUNK, 128, L).transpose(1, 0, 2).reshape(128, NCHUNK * L))
        wmask = (tw[None, :] * valid.astype(np.float32))   # [512, 50]
        wmask = np.ascontiguousarray(
            wmask.reshape(NCHUNK, 128, L).transpose(1, 0, 2).reshape(128, NCHUNK * L))
        in_maps.append({
            "table": table,
            "identin": ident_np,
            "utable": utable,
            "wproj": wproj,
            "beff": beff,
            "bmask": bmask,
            "idx": idx,
            "wm": wmask.astype(np.float32),
            "uid": np.ascontiguousarray(uid_all[s]),
            "prefin": np.ascontiguousarray(pref_all[s]),
            "posT": np.ascontiguousarray(pos_all[s].reshape(1, BK)),
            "negT": np.ascontiguousarray(neg_all[s].reshape(1, BK)),
        })
    return in_maps


def _reduce_outputs(results):
    bce_total = 0.0
    div_total = 0.0
    for r in results:
        o = np.asarray(r["out"], dtype=np.float64)
        bce_total += o[:BLOC].sum()
        div_total += o[BLOC:].sum()
    loss = bce_total + DIV_TRADEOFF * (2.0 * div_total) / (B * K * K)
    return np.asarray(loss, dtype=np.float32)


def kernel(**inputs):
    nc = _get_nc()
    in_maps = _prep_in_maps(**inputs)
    res = run_bass_kernel_spmd(nc, in_maps, list(range(NCORES)))
    return _reduce_outputs(res.results)



# revision 7
# speedup vs baseline: 1.1170x; 1.1170x over previous
"""EnsRec loss kernel for 8 Trainium2 NeuronCores.

Data-parallel over batch (64 rows per core); item/user tables and W_proj
replicated. Algebraic restructuring vs the reference:
  basemodel_emb = (sum_l tw[l]*mask*E[ids]) @ W_proj + b_proj*sum(tw)
(projection commutes with the time-decay sum, and the validity mask is
folded into the per-hit weight so id==0 rows need no table edit).

v3: the 25600 per-core table-row gathers are issued as a handful of
batched `dma_gather` instructions (one per (bk-chunk, 32768-row table
range); ids are range-split so the int16 index constraint holds), and
the weighted accumulate runs on the TensorEngine as one-hot scatter
matmuls (acc += S_j^T @ G_j in PSUM) built on DVE from an iota tile.
This removes both baseline bottlenecks: the ~1us/call SWDGE fixed cost
(200 indirect DMAs) and the ~1us/op DVE accumulate chain (200 ops).

Each core emits per-row partial losses; the host does the final 8-way sum.
"""

import sys

import numpy as np

_TRN_REPO = "/opt/trn_rl_repo"
if _TRN_REPO not in sys.path:
    sys.path.insert(0, _TRN_REPO)

import concourse.bacc as bacc
import concourse.mybir as mybir
import concourse.tile as tile
from concourse.bass import IndirectOffsetOnAxis
from concourse.bass_utils import run_bass_kernel_spmd

B, K, L, D, H = 512, 8, 50, 768, 128
N_ITEM = 100000
N_USER = 50000
DIV_TRADEOFF = 0.1
NCORES = 8
BLOC = B // NCORES          # 64 batch rows per core
BK = BLOC * K               # 512 (b,k) rows per core
NCHUNK = BK // 128          # 4 partition-tiles of (b,k) rows
RB = 15                     # table split into 2**RB-row ranges for int16 idx
NRANGE = (N_ITEM >> RB) + 1
OUT_LEN = 2 * BLOC

_f32 = mybir.dt.float32
_bf16 = mybir.dt.bfloat16
_i32 = mybir.dt.int32
_i16 = mybir.dt.int16
ALU = mybir.AluOpType
AFT = mybir.ActivationFunctionType
AXL = mybir.AxisListType

_CACHED = {}


def _build_module(cols, tot_cols, gcap):
    """cols[t][m] = gather columns (128 hits each) for chunk t, range m."""
    nc = bacc.Bacc("TRN2", target_bir_lowering=False, debug=False,
                   num_devices=NCORES)

    table = nc.dram_tensor("table", [N_ITEM + 1, D], _bf16, kind="ExternalInput")
    utable = nc.dram_tensor("utable", [N_USER, H], _f32, kind="ExternalInput")
    wproj = nc.dram_tensor("wproj", [D, H], _f32, kind="ExternalInput")
    beff = nc.dram_tensor("beff", [H, 1], _f32, kind="ExternalInput")
    bmask = nc.dram_tensor("bmask", [128, 128], _f32, kind="ExternalInput")
    identin = nc.dram_tensor("identin", [128, 128], _f32, kind="ExternalInput")
    iotain = nc.dram_tensor("iotain", [128, 128], _bf16, kind="ExternalInput")
    idx16 = nc.dram_tensor("idx16", [128, tot_cols * 8], _i16,
                           kind="ExternalInput")
    wcol = nc.dram_tensor("wcol", [128, tot_cols], _f32, kind="ExternalInput")
    rcol = nc.dram_tensor("rcol", [128, tot_cols], _f32, kind="ExternalInput")
    uid = nc.dram_tensor("uid", [BLOC, 1], _i32, kind="ExternalInput")
    prefin = nc.dram_tensor("prefin", [BLOC, H], _f32, kind="ExternalInput")
    posT = nc.dram_tensor("posT", [1, BK], _f32, kind="ExternalInput")
    negT = nc.dram_tensor("negT", [1, BK], _f32, kind="ExternalInput")
    out = nc.dram_tensor("out", [OUT_LEN], _f32, kind="ExternalOutput")
    wscr1 = nc.dram_tensor("wscr1", [BK], _f32)

    with tile.TileContext(nc) as tc:
        with (
            tc.tile_pool(name="gp", bufs=3) as gp,
            tc.tile_pool(name="spl", bufs=4) as spool,
            tc.tile_pool(name="sb", bufs=1) as sbp,
            tc.tile_pool(name="work", bufs=2) as workp,
            tc.tile_pool(name="pacc", bufs=1, space="PSUM") as pacc,
            tc.tile_pool(name="ps2", bufs=2, space="PSUM") as ps2,
            tc.tile_pool(name="ps1", bufs=1, space="PSUM") as ps1,
        ):
            ident = sbp.tile([128, 128], _f32, tag="ident")
            nc.sync.dma_start(out=ident[:], in_=identin[:])
            iota_sb = sbp.tile([128, 128], _bf16, tag="iota")
            nc.sync.dma_start(out=iota_sb[:], in_=iotain[:])
            idx_sb = sbp.tile([128, tot_cols * 8], _i16, tag="idx")
            nc.sync.dma_start(out=idx_sb[:], in_=idx16[:])
            wcol_sb = sbp.tile([128, tot_cols], _f32, tag="wcol")
            nc.sync.dma_start(out=wcol_sb[:], in_=wcol[:])
            rcol_sb = sbp.tile([128, tot_cols], _f32, tag="rcol")
            nc.sync.dma_start(out=rcol_sb[:], in_=rcol[:])
            wall = sbp.tile([128, 6 * 128], _f32, tag="wall")
            for c in range(6):
                nc.sync.dma_start(out=wall[:, c * 128:(c + 1) * 128],
                                  in_=wproj[c * 128:(c + 1) * 128, :])
            beff_sb = sbp.tile([H, 1], _f32, tag="beff")
            nc.sync.dma_start(out=beff_sb[:], in_=beff[:])
            bmask_sb = sbp.tile([128, 128], _f32, tag="bmask")
            nc.sync.dma_start(out=bmask_sb[:], in_=bmask[:])

            # ---- preference = prefin + utable[uid], transposed+replicated ----
            uid_sb = sbp.tile([BLOC, 1], _i32, tag="uid")
            nc.sync.dma_start(out=uid_sb[:], in_=uid[:])
            pref = sbp.tile([BLOC, H], _f32, tag="pref")
            nc.gpsimd.indirect_dma_start(
                out=pref[:], out_offset=None, in_=utable[:],
                in_offset=IndirectOffsetOnAxis(ap=uid_sb[:, :1], axis=0))
            prefin_sb = sbp.tile([BLOC, H], _f32, tag="prefin")
            nc.sync.dma_start(out=prefin_sb[:], in_=prefin[:])
            nc.vector.tensor_tensor(out=pref[:], in0=pref[:], in1=prefin_sb[:],
                                    op=ALU.add)
            ptp = ps1.tile([128, BLOC], _f32, tag="ptp")
            nc.tensor.transpose(out=ptp[:], in_=pref[:],
                                identity=ident[:BLOC, :BLOC])
            prep = sbp.tile([128, 512], _f32, tag="prep")
            prep3 = prep[:].rearrange("p (b k) -> p b k", k=K)
            for k in range(K):
                nc.vector.tensor_copy(out=prep3[:, :, k], in_=ptp[:])

            # ---- main gather + PE scatter-accumulate ----
            # dma_gather writes hit i of a group to [i%128, i//128, :]; the
            # DVE builds S_j[p, r] = w(hit) * [r == target row of hit] from a
            # constant iota tile, and the TensorEngine accumulates
            # acc += S_j^T @ G_j in PSUM across all of a chunk's columns.
            wsumT = sbp.tile([128, 6 * 512], _f32, tag="wsumT")
            eT = sbp.tile([128, 512], _f32, tag="eT")
            r_all = sbp.tile([128, NCHUNK], _f32, tag="rall")
            ones = sbp.tile([128, 1], _f32, tag="ones")
            nc.vector.memset(ones[:], 1.0)
            wop = ps1.tile([1, 512], _f32, tag="wop")
            coff = 0
            for t in range(NCHUNK):
                accA = pacc.tile([128, 384], _f32, tag="accA")
                accB = pacc.tile([128, 384], _f32, tag="accB")
                tcols = sum(cols[t])
                jg = 0
                for m in range(NRANGE):
                    gcols = cols[t][m]
                    if gcols == 0:
                        continue
                    rbase = m << RB
                    rlen = min(N_ITEM + 1 - rbase, 1 << RB)
                    gt = gp.tile([128, gcap * D], _bf16, tag="gath")
                    gt3 = gt[:].rearrange("p (j d) -> p j d", d=D)
                    nc.gpsimd.dma_gather(
                        out_ap=gt3[:, 0:gcols, :],
                        in_ap=table[rbase:rbase + rlen],
                        idxs_ap=idx_sb[:, (coff + jg) * 8:(coff + jg + gcols) * 8],
                        num_idxs=128 * gcols,
                        num_idxs_reg=128 * gcols,
                        elem_size=D,
                        single_packet=False,
                    )
                    for jl in range(gcols):
                        j = jg + jl
                        S = spool.tile([128, 128], _bf16, tag="S")
                        nc.vector.tensor_scalar(
                            out=S[:], in0=iota_sb[:],
                            scalar1=rcol_sb[:, coff + j:coff + j + 1],
                            scalar2=wcol_sb[:, coff + j:coff + j + 1],
                            op0=ALU.is_equal, op1=ALU.mult)
                        nc.tensor.matmul(out=accA[:], lhsT=S[:],
                                         rhs=gt3[:, jl, 0:384],
                                         start=(j == 0), stop=(j == tcols - 1))
                        nc.tensor.matmul(out=accB[:], lhsT=S[:],
                                         rhs=gt3[:, jl, 384:768],
                                         start=(j == 0), stop=(j == tcols - 1))
                    jg += gcols
                coff += tcols
                acc = workp.tile([128, D], _f32, tag="acc")
                nc.scalar.activation(out=acc[:, 0:384], in_=accA[:],
                                     func=AFT.Copy)
                nc.scalar.activation(out=acc[:, 384:768], in_=accB[:],
                                     func=AFT.Copy)
                # per-chunk tail: transpose, project, gram, score
                for c in range(6):
                    tp = ps2.tile([128, 128], _f32, tag="tp")
                    nc.tensor.transpose(out=tp[:],
                                        in_=acc[:, c * 128:(c + 1) * 128],
                                        identity=ident[:])
                    nc.vector.tensor_copy(
                        out=wsumT[:, c * 512 + t * 128: c * 512 + (t + 1) * 128],
                        in_=tp[:])
                eTp = ps2.tile([128, 128], _f32, tag="eTp", bufs=1)
                for c in range(6):
                    nc.tensor.matmul(
                        out=eTp[:],
                        lhsT=wall[:, c * 128:(c + 1) * 128],
                        rhs=wsumT[:, c * 512 + t * 128: c * 512 + (t + 1) * 128],
                        start=(c == 0), stop=(c == 5))
                nc.vector.tensor_scalar(out=eT[:, t * 128:(t + 1) * 128],
                                        in0=eTp[:], scalar1=beff_sb[:],
                                        scalar2=None, op0=ALU.add)
                sp = ps2.tile([128, 128], _f32, tag="sp", bufs=1)
                nc.tensor.matmul(out=sp[:], lhsT=eT[:, t * 128:(t + 1) * 128],
                                 rhs=eT[:, t * 128:(t + 1) * 128],
                                 start=True, stop=True)
                spc = workp.tile([128, 128], _f32, tag="spc")
                nc.vector.tensor_copy(out=spc[:], in_=sp[:])
                s2 = workp.tile([128, 128], _f32, tag="s2")
                nc.vector.tensor_tensor(out=s2[:], in0=sp[:], in1=spc[:],
                                        op=ALU.mult)
                dummy = workp.tile([128, 128], _f32, tag="dummy")
                nc.vector.scalar_tensor_tensor(
                    out=dummy[:], in0=s2[:], scalar=1.0, in1=bmask_sb[:],
                    op0=ALU.mult, op1=ALU.mult, accum_out=r_all[:, t:t + 1])
                prod = workp.tile([128, 128], _f32, tag="prod")
                nc.vector.tensor_tensor(out=prod[:],
                                        in0=eT[:, t * 128:(t + 1) * 128],
                                        in1=prep[:, t * 128:(t + 1) * 128],
                                        op=ALU.mult)
                nc.tensor.matmul(out=wop[:, t * 128:(t + 1) * 128],
                                 lhsT=ones[:], rhs=prod[:],
                                 start=True, stop=True)

            # ---- tail entirely in the [1, BK] row layout (col = b*K + k) ----
            # u = exp(worg) (no max-sub: |worg| <~ 6); softmax normalization is
            # folded into the pu/nu and div ratios, so no k-broadcast needed.
            u = sbp.tile([1, BK], _f32, tag="u")
            nc.scalar.activation(out=u[:], in_=wop[:], func=AFT.Exp)
            u3 = u[:].rearrange("o (b k) -> o b k", k=K)
            s = sbp.tile([1, BLOC], _f32, tag="s")
            nc.vector.tensor_reduce(out=s[:], in_=u3, axis=AXL.X, op=ALU.add)
            rs = sbp.tile([1, BLOC], _f32, tag="rs")
            nc.vector.reciprocal(out=rs[:], in_=s[:])

            pos_sb = sbp.tile([1, BK], _f32, tag="pos")
            nc.sync.dma_start(out=pos_sb[:], in_=posT[:])
            neg_sb = sbp.tile([1, BK], _f32, tag="neg")
            nc.sync.dma_start(out=neg_sb[:], in_=negT[:])
            pu = sbp.tile([1, BK], _f32, tag="pu")
            nc.vector.tensor_tensor(out=pu[:], in0=pos_sb[:], in1=u[:],
                                    op=ALU.mult)
            pug = sbp.tile([1, BLOC], _f32, tag="pug")
            nc.vector.tensor_reduce(out=pug[:],
                                    in_=pu[:].rearrange("o (b k) -> o b k", k=K),
                                    axis=AXL.X, op=ALU.add)
            nu = sbp.tile([1, BK], _f32, tag="nu")
            nc.vector.tensor_tensor(out=nu[:], in0=neg_sb[:], in1=u[:],
                                    op=ALU.mult)
            nug = sbp.tile([1, BLOC], _f32, tag="nug")
            nc.vector.tensor_reduce(out=nug[:],
                                    in_=nu[:].rearrange("o (b k) -> o b k", k=K),
                                    axis=AXL.X, op=ALU.add)
            dnum = sbp.tile([1, BLOC], _f32, tag="dnum")
            nc.vector.tensor_tensor(out=dnum[:], in0=pug[:], in1=nug[:],
                                    op=ALU.subtract)
            dlt = sbp.tile([1, BLOC], _f32, tag="dlt")
            nc.vector.tensor_tensor(out=dlt[:], in0=dnum[:], in1=rs[:],
                                    op=ALU.mult)
            expt = sbp.tile([1, BLOC], _f32, tag="expt")
            nc.scalar.activation(out=expt[:], in_=dlt[:], func=AFT.Exp,
                                 scale=-1.0)
            bce = sbp.tile([1, BLOC], _f32, tag="bce")
            nc.scalar.activation(out=bce[:], in_=expt[:], func=AFT.Ln,
                                 bias=1.0)
            nc.sync.dma_start(out=out[None, 0:BLOC], in_=bce[:])

            # ---- div part: bounce r_all to the row layout, then
            # out[64:128] = per-b sum_k u*r / s ----
            rtp = ps1.tile([NCHUNK, 128], _f32, tag="ptp")
            nc.tensor.transpose(out=rtp[:], in_=r_all[:], identity=ident[:])
            rts = sbp.tile([NCHUNK, 128], _f32, tag="rts")
            nc.vector.tensor_copy(out=rts[:], in_=rtp[:])
            nc.scalar.dma_start(out=wscr1[:].rearrange("(t p) -> t p", p=128),
                                in_=rts[:])
            rrow = sbp.tile([1, BK], _f32, tag="rrow")
            nc.scalar.dma_start(out=rrow[:], in_=wscr1[None, :])
            ur = sbp.tile([1, BK], _f32, tag="ur")
            nc.vector.tensor_tensor(out=ur[:], in0=u[:], in1=rrow[:],
                                    op=ALU.mult)
            urg = sbp.tile([1, BLOC], _f32, tag="urg")
            nc.vector.tensor_reduce(out=urg[:],
                                    in_=ur[:].rearrange("o (b k) -> o b k", k=K),
                                    axis=AXL.X, op=ALU.add)
            dvb = sbp.tile([1, BLOC], _f32, tag="dvb")
            nc.vector.tensor_tensor(out=dvb[:], in0=urg[:], in1=rs[:],
                                    op=ALU.mult)
            nc.sync.dma_start(out=out[None, BLOC:], in_=dvb[:])

    nc.compile()
    return nc


def _get_nc(cols, tot_cols, gcap):
    key = (tuple(map(tuple, cols)), tot_cols, gcap)
    if key not in _CACHED:
        _CACHED[key] = _build_module(cols, tot_cols, gcap)
    return _CACHED[key]


def _prep_in_maps(user_id, base_model_preds, preference_in, pos_label,
                  neg_label, user_embeddings, item_embeddings, W_proj, b_proj):
    tw = (1.0 / np.log2(np.arange(L, dtype=np.float32) + 2.0)).astype(np.float32)
    import ml_dtypes
    table = np.ascontiguousarray(
        np.asarray(item_embeddings, dtype=np.float32).astype(ml_dtypes.bfloat16))
    utable = np.ascontiguousarray(np.asarray(user_embeddings, dtype=np.float32))
    wproj = np.ascontiguousarray(np.asarray(W_proj, dtype=np.float32))
    beff = (np.asarray(b_proj, dtype=np.float32) * np.float32(tw.sum())
            ).reshape(H, 1)
    ident_np = np.eye(128, dtype=np.float32)
    iota_np = np.broadcast_to(np.arange(128, dtype=np.float32), (128, 128))
    iota_np = np.ascontiguousarray(iota_np.astype(ml_dtypes.bfloat16))
    bmask = (np.kron(np.eye(16, dtype=np.float32),
                     np.ones((8, 8), dtype=np.float32))
             - np.eye(128, dtype=np.float32)).astype(np.float32)

    preds = np.asarray(base_model_preds).astype(np.int64)
    uid_all = np.asarray(user_id).astype(np.int32).reshape(B, 1)
    pref_all = np.asarray(preference_in, dtype=np.float32)
    pos_all = np.asarray(pos_label, dtype=np.float32)
    neg_all = np.asarray(neg_label, dtype=np.float32)

    # ---- per (core, chunk, range): range-sorted padded hit groups ----
    # hit = (target row r in chunk, l); groups padded to 128-multiples with
    # (local_id=0, r=0, w=0) so every gathered slot holds finite table data.
    per_core = []
    raw_cols = np.zeros((NCORES, NCHUNK, NRANGE), dtype=np.int64)
    for c in range(NCORES):
        s = slice(c * BLOC, (c + 1) * BLOC)
        pf = preds[s].reshape(BK, L)
        valid = (pf > 0) & (pf <= N_ITEM)
        safe = np.where(valid, pf, 0).astype(np.int64)
        wfull = tw[None, :] * valid.astype(np.float32)
        chunks = []
        for t in range(NCHUNK):
            ids = safe[t * 128:(t + 1) * 128].reshape(-1)     # r-major flat
            ws = wfull[t * 128:(t + 1) * 128].reshape(-1)
            rr = np.repeat(np.arange(128), L)
            m = ids >> RB
            groups = []
            for mm in range(NRANGE):
                sel = np.nonzero(m == mm)[0]
                raw_cols[c, t, mm] = (len(sel) + 127) // 128
                groups.append((ids[sel] - (mm << RB), rr[sel], ws[sel]))
            chunks.append(groups)
        per_core.append(chunks)
    # uniform column counts across cores (one shared SPMD module)
    cols = tuple(tuple(int(raw_cols[:, t, mm].max()) for mm in range(NRANGE))
                 for t in range(NCHUNK))
    tot_cols = int(sum(sum(ct) for ct in cols))
    gcap = int(max(max(ct) for ct in cols))

    in_maps = []
    for c in range(NCORES):
        idx16_np = np.zeros((128, tot_cols * 8), dtype=np.int16)
        wcol_np = np.zeros((128, tot_cols), dtype=np.float32)
        rcol_np = np.zeros((128, tot_cols), dtype=np.float32)
        coff = 0
        for t in range(NCHUNK):
            for mm in range(NRANGE):
                gcols = cols[t][mm]
                if gcols == 0:
                    continue
                lids, rrs, wss = per_core[c][t][mm]
                n = 128 * gcols
                lid_p = np.zeros(n, dtype=np.int16)
                r_p = np.zeros(n, dtype=np.float32)
                w_p = np.zeros(n, dtype=np.float32)
                lid_p[:len(lids)] = lids
                r_p[:len(lids)] = rrs
                w_p[:len(lids)] = wss
                # idxs wrapped: hit i at [i%16, i//16], replicated to 128 rows
                iw = lid_p.reshape(gcols * 8, 16).T            # [16, 8*gcols]
                idx16_np[:, coff * 8:(coff + gcols) * 8] = np.tile(iw, (8, 1))
                # gathered layout: hit i -> [i%128, i//128]
                wcol_np[:, coff:coff + gcols] = w_p.reshape(gcols, 128).T
                rcol_np[:, coff:coff + gcols] = r_p.reshape(gcols, 128).T
                coff += gcols
        s = slice(c * BLOC, (c + 1) * BLOC)
        in_maps.append({
            "table": table,
            "identin": ident_np,
            "iotain": iota_np,
            "utable": utable,
            "wproj": wproj,
            "beff": beff,
            "bmask": bmask,
            "idx16": idx16_np,
            "wcol": wcol_np,
            "rcol": rcol_np,
            "uid": np.ascontiguousarray(uid_all[s]),
            "prefin": np.ascontiguousarray(pref_all[s]),
            "posT": np.ascontiguousarray(pos_all[s].reshape(1, BK)),
            "negT": np.ascontiguousarray(neg_all[s].reshape(1, BK)),
        })
    return in_maps, cols, tot_cols, gcap


def _reduce_outputs(results):
    bce_total = 0.0
    div_total = 0.0
    for r in results:
        o = np.asarray(r["out"], dtype=np.float64)
        bce_total += o[:BLOC].sum()
        div_total += o[BLOC:].sum()
    loss = bce_total + DIV_TRADEOFF * (2.0 * div_total) / (B * K * K)
    return np.asarray(loss, dtype=np.float32)


def prepare(inputs):
    in_maps, cols, tot_cols, gcap = _prep_in_maps(**inputs)
    nc = _get_nc(cols, tot_cols, gcap)
    return nc, in_maps


def kernel(**inputs):
    nc, in_maps = prepare(inputs)
    res = run_bass_kernel_spmd(nc, in_maps, list(range(NCORES)))
    return _reduce_outputs(res.results)
